# revision 1
# baseline (speedup 1.0000x reference)
"""Trainium2 Bass kernel for nn_DifferentiableHodgeProxy.

Self-contained. Shards the batch over 8 NeuronCores; each core runs a raw-Bass
(explicit semaphore) program emitted through a small dependency-tracking
scheduler (Prog).

Math (validated vs the jax reference in numpy, rel err ~9e-4):
  spec(L1) - tau = [spec(Mt) minus one zero] U spec(Ut)   since B1 @ B2 = 0
    Mt = sqrt(act)sqrt(act)^T * (K I - 11^T)       16x16 per sample
    Ut = C diag(W2) C^T, C = V^T B2 (V = onb of im B2)   105x105 per (s,b)
  Eigenvalues via batched Householder tridiagonalization + Sturm bisection
  (only the smallest 4/5 needed). L0 (16x16) handled the same way.
"""
import numpy as np
from contextlib import ExitStack

from concourse import bass, mybir
from concourse.bass_utils import run_bass_kernel_spmd

f32 = mybir.dt.float32
i32 = mybir.dt.int32
AF = mybir.ActivationFunctionType
OP = mybir.AluOpType
AX = mybir.AxisListType

MAXP, S, J, TAU, HID, LIFT = 16, 3, 4, 1e-4, 256, 16
K = MAXP
E, T, R = 120, 560, 105
B_BATCH, N_PTS = 256, 4096
NCORES = 8
BC = B_BATCH // NCORES        # 32 samples/core
NPROB = S * BC                # 96 problems/core, s-major: p = 32*s + b
M16 = 128                     # 96 L0 + 32 Mt sixteen-dim problems
BIS_IT16 = 6
BIS_IT105 = 6
MU_SLAB = 512


# ------------------------------------------------------------ mini scheduler
class Prog:
    """Raw-bass emitter: records ops per engine, computes cross-engine waits
    (vector clocks -> standalone wait_ge) and same-engine drains."""

    ENGINES = ("sync", "vector", "scalar", "tensor", "gpsimd")
    DRAIN_ENGINES = ("vector", "scalar", "gpsimd")

    def __init__(self, nc):
        self.nc = nc
        self.ops = []
        self.writer = {}
        self.readers = {}
        self.tick = {e: 0 for e in self.ENGINES}
        self.dma_tick = {"sync": 0, "gpsimd": 0, "scalar": 0}

    @staticmethod
    def _names(aps):
        out = []
        for a in aps:
            if isinstance(a, str):
                out.append(a)
                continue
            t = a.tensor if isinstance(a, bass.AP) else a
            out.append(t.name)
        return out

    def op(self, engine, emit, reads=(), writes=()):
        self.ops.append((engine, emit, self._names(reads), self._names(writes),
                         None))

    def dma(self, engine, out_ap, in_ap):
        def emit(eng):
            return eng.dma_start(out=out_ap, in_=in_ap)
        self.ops.append((engine, emit, self._names([in_ap]),
                         self._names([out_ap]), "dma"))

    def indirect(self, out_ap, in_ap, off_ap):
        def emit(eng):
            return eng.indirect_dma_start(
                out=out_ap, out_offset=None, in_=in_ap,
                in_offset=bass.IndirectOffsetOnAxis(ap=off_ap, axis=0))
        self.ops.append(("gpsimd", emit, self._names([in_ap, off_ap]),
                         self._names([out_ap]), "dma"))

    def build(self):
        nc = self.nc
        plans = []
        observed = {e: {} for e in self.ENGINES}
        last_drain = {e: 0 for e in self.ENGINES}

        def need(engine, waits, semkey, tick):
            if observed[engine].get(semkey, 0) < tick:
                waits[semkey] = max(waits.get(semkey, 0), tick)

        for engine, emit, reads, writes, dma in self.ops:
            waits = {}
            same_dep = 0
            mykey = ("dma_" + engine) if dma == "dma" else engine
            for rname in reads:
                for wkey, wtick in self.writer.get(rname, {}).items():
                    if wkey != mykey:
                        need(engine, waits, wkey, wtick)
                    else:
                        same_dep = max(same_dep, wtick)
            for wname in writes:
                for wkey, wtick in self.writer.get(wname, {}).items():
                    if wkey != mykey:
                        need(engine, waits, wkey, wtick)
                    else:
                        same_dep = max(same_dep, wtick)
                for reng, rtick in self.readers.get(wname, {}).items():
                    if reng != mykey:
                        need(engine, waits, reng, rtick)
                    else:
                        same_dep = max(same_dep, rtick)
            drain_before = (engine in self.DRAIN_ENGINES and dma != "dma"
                            and same_dep > last_drain[engine]
                            and same_dep > self.tick[engine] - 2)
            for semkey, tick in waits.items():
                observed[engine][semkey] = tick
            if dma == "dma":
                self.dma_tick[engine] += 16
                wtick = ("dma_" + engine, self.dma_tick[engine])
            else:
                self.tick[engine] += 1
                wtick = (engine, self.tick[engine])
            if drain_before:
                last_drain[engine] = self.tick[engine] - (0 if dma else 1)
            plans.append((engine, emit, waits, wtick, drain_before, dma))
            for rname in reads:
                self.readers.setdefault(rname, {})[wtick[0]] = wtick[1]
            for wname in writes:
                self.writer.setdefault(wname, {})[wtick[0]] = wtick[1]
                self.readers[wname] = {}

        semnames = ["sync", "vector", "scalar", "tensor", "gpsimd",
                    "dma_sync", "dma_gpsimd", "dma_scalar"]
        ctx = ExitStack()
        sems = {n: ctx.enter_context(nc.semaphore("sem_" + n)) for n in semnames}
        per_engine = {e: [] for e in self.ENGINES}
        for engine, emit, waits, wtick, drain_before, dma in plans:
            per_engine[engine].append((emit, waits, wtick, drain_before, dma))

        with nc.Block() as block:
            for ename in self.ENGINES:
                items = per_engine[ename]
                if not items:
                    continue

                def make_body(items):
                    def body(eng):
                        for emit, waits, wtick, drain_before, dma in items:
                            if drain_before:
                                eng.drain()
                            for semkey, tick in sorted(waits.items()):
                                eng.wait_ge(sems[semkey], tick)
                            inst = emit(eng)
                            inst.then_inc(sems[wtick[0]], 16 if dma else 1)
                    return body

                getattr(block, ename)(make_body(items))
        ctx.close()


# ---------------------------------------------------------- host-side consts
def _build_complex():
    edges = [(i, j) for i in range(K) for j in range(i + 1, K)]
    tris = [(i, j, k) for i in range(K) for j in range(i + 1, K)
            for k in range(j + 1, K)]
    B1 = np.zeros((K, E), np.float64)
    for e, (i, j) in enumerate(edges):
        B1[i, e] = -1.0
        B1[j, e] = 1.0
    e2i = {e: n for n, e in enumerate(edges)}
    B2 = np.zeros((E, T), np.float64)
    for t, (i, j, k) in enumerate(tris):
        B2[e2i[(j, k)], t] = 1.0
        B2[e2i[(i, k)], t] = -1.0
        B2[e2i[(i, j)], t] = 1.0
    return edges, tris, e2i, B1, B2


_CC = {}


def host_constants():
    if _CC:
        return _CC
    edges, tris, e2i, B1, B2 = _build_complex()
    U, s, _ = np.linalg.svd(B2, full_matrices=False)
    V = U[:, :R]
    C = V.T @ B2
    Mu = np.einsum('et,ft->tef', C, C).reshape(T, R * R).astype(np.float32)
    Se = np.zeros((256, E), np.float32)
    for e, (i, j) in enumerate(edges):
        Se[i * K + j, e] = 1.0
    G = np.zeros((3, E, T), np.float32)
    for t, (i, j, k) in enumerate(tris):
        G[0, e2i[(i, j)], t] = 1.0
        G[1, e2i[(j, k)], t] = 1.0
        G[2, e2i[(i, k)], t] = 1.0
    M0T = np.einsum('ke,le->kle', B1, B1).reshape(256, E).T.copy().astype(np.float32)
    patt = np.tile((K * np.eye(K) - np.ones((K, K))).reshape(1, 256), (BC, 1)
                   ).astype(np.float32)
    tri_m = np.tile(np.triu(np.ones((K, K)), 1).reshape(1, 256), (BC, 1)
                    ).astype(np.float32)
    id128 = np.eye(128, dtype=np.float32)
    base = np.tile(np.arange(BC, dtype=np.int64)[:, None] * N_PTS - 1,
                   (1, K)).astype(np.int32)
    rr16 = np.tile(np.arange(1, 9, dtype=np.float32)[None, None, :],
                   (M16, 5, 1)).reshape(M16, 40)
    jt16 = np.tile(np.arange(1, 6, dtype=np.float32)[None, :, None],
                   (M16, 1, 8)).reshape(M16, 40)
    rr105 = np.tile(np.arange(1, 9, dtype=np.float32)[None, None, :],
                    (NPROB, 4, 1)).reshape(NPROB, 32)
    jt105 = np.tile(np.arange(1, 5, dtype=np.float32)[None, :, None],
                    (NPROB, 1, 8)).reshape(NPROB, 32)
    _CC.update(dict(Mu=Mu, Se=Se, G=G, M0T=M0T, patt=patt, tri=tri_m,
                    id128=id128, base=base, rr16=rr16, jt16=jt16,
                    rr105=rr105, jt105=jt105))
    return _CC


# ------------------------------------------------------------- eig emitters
def emit_tridiag(pg, A_t, scr_t, vb_t, av_t, pb_t, qb_t, eb_t, sm, Pp, m,
                 split=False):
    """Householder tridiag of (Pp, m*m) flat symmetric batch.
    split=True row-splits the O(L^2) ops between DVE (top) and GPSIMD
    (bottom); soundness: each step's first DVE op reads both regions."""
    A = A_t[:Pp, 0:m * m]
    scr = scr_t[:Pp, 0:(m - 1) * (m - 1)]
    AL, AH = A_t.name + "#lo", A_t.name + "#hi"
    SL, SH = scr_t.name + "#lo", scr_t.name + "#hi"
    for k in range(m - 2):
        L = m - 1 - k
        x = A[:, k * m + k + 1: k * m + k + 1 + L]
        t_l = scr[:, 0:L]
        # s2 = sum(x*x) fused
        pg.op("vector", lambda e, x=x, t=t_l: e.scalar_tensor_tensor(
            out=t, in0=x, scalar=1.0, in1=x, op0=OP.mult, op1=OP.mult,
            accum_out=sm["s2"][:]) if False else e.scalar_tensor_tensor(
            out=t, in0=x, scalar=1.0, in1=x, op0=OP.bypass, op1=OP.mult,
            accum_out=sm["s2"][:]),
            reads=([AL, AH, A_t.name, scr_t.name] if k == 0 else [AL, AH]),
            writes=[SL, sm["s2"]])
        pg.op("scalar", lambda e: e.sqrt(out=sm["sig"][:], in_=sm["s2"][:]),
              reads=[sm["s2"]], writes=[sm["sig"]])
        pg.op("vector", lambda e, x=x: e.tensor_scalar_add(
            out=sm["x0"][:], in0=x[:, 0:1], scalar1=1e-30),
            reads=[AL], writes=[sm["x0"]])
        pg.op("scalar", lambda e: e.sign(out=sm["sgn"][:], in_=sm["x0"][:]),
              reads=[sm["x0"]], writes=[sm["sgn"]])
        # al = -(sgn*sig) fused
        pg.op("vector", lambda e: e.scalar_tensor_tensor(
            out=sm["al"][:], in0=sm["sgn"][:], scalar=-1.0, in1=sm["sig"][:],
            op0=OP.mult, op1=OP.mult),
            reads=[sm["sgn"], sm["sig"]], writes=[sm["al"]])
        v = vb_t[:Pp, 0:L]
        pg.op("scalar", lambda e, x=x, v=v: e.copy(out=v, in_=x),
              reads=[AL], writes=[vb_t])
        pg.op("vector", lambda e, v=v: e.tensor_tensor(
            out=v[:, 0:1], in0=v[:, 0:1], in1=sm["al"][:], op=OP.subtract),
            reads=[vb_t, sm["al"]], writes=[vb_t])
        # vtv = sum(v*v) fused (into t_l, reusing scr lo region)
        pg.op("vector", lambda e, v=v, t=t_l: e.scalar_tensor_tensor(
            out=t, in0=v, scalar=1.0, in1=v, op0=OP.bypass, op1=OP.mult,
            accum_out=sm["vtv"][:]),
            reads=[vb_t], writes=[SL, sm["vtv"]])
        pg.op("vector", lambda e: e.tensor_scalar_add(
            out=sm["vtv"][:], in0=sm["vtv"][:], scalar1=1e-30),
            reads=[sm["vtv"]], writes=[sm["vtv"]])
        pg.op("vector", lambda e: e.reciprocal(out=sm["r"][:], in_=sm["vtv"][:]),
              reads=[sm["vtv"]], writes=[sm["r"]])
        pg.op("vector", lambda e: e.tensor_scalar_mul(
            out=sm["r2"][:], in0=sm["r"][:], scalar1=2.0),
            reads=[sm["r"]], writes=[sm["r2"]])
        # matvec: split rows [0:Ls) on DVE, [Ls:L) on GPSIMD
        Ls = min(L, max(1, (L * 4 + 4) // 5)) if split else L
        base = (k + 1) * m + (k + 1)
        prt = scr[:, 0:L * L]

        def mrows(r0, r1, base=base):
            return bass.AP(A.tensor, A.offset + base + r0 * m,
                           [list(A.ap[0]), [m, r1 - r0], [1, L]])

        def srows(r0, r1):
            return bass.AP(prt.tensor, prt.offset + r0 * L,
                           [list(prt.ap[0]), [L, r1 - r0], [1, L]])

        vrow_n = lambda n: v.unsqueeze(1).to_broadcast([Pp, n, L])
        a_mv_o, a_mv_i, a_mv_v = srows(0, Ls), mrows(0, Ls), vrow_n(Ls)
        pg.op("vector", lambda e, a=a_mv_o, b=a_mv_i, c=a_mv_v: e.tensor_mul(
            out=a, in0=b, in1=c),
            reads=[AL, AH, vb_t], writes=[SL])
        if split and Ls < L:
            g_mv_o, g_mv_i, g_mv_v = srows(Ls, L), mrows(Ls, L), vrow_n(L - Ls)
            pg.op("gpsimd", lambda e, a=g_mv_o, b=g_mv_i, c=g_mv_v:
                  e.tensor_mul(out=a, in0=b, in1=c),
                  reads=[AL, AH, vb_t], writes=[SH])
        a_rd_i, a_rd_o = srows(0, L), av_t[:Pp, 0:L]
        pg.op("vector", lambda e, a=a_rd_o, b=a_rd_i: e.tensor_reduce(
            out=a, in_=b, axis=AX.X, op=OP.add),
            reads=[SL, SH], writes=[av_t])
        pg.op("vector", lambda e, L=L: e.tensor_scalar_mul(
            out=pb_t[:Pp, 0:L], in0=av_t[:Pp, 0:L], scalar1=sm["r2"][:]),
            reads=[av_t, sm["r2"]], writes=[pb_t])
        # pv = sum(p*v) fused
        pg.op("vector", lambda e, v=v, L=L, t=t_l: e.scalar_tensor_tensor(
            out=t, in0=pb_t[:Pp, 0:L], scalar=1.0, in1=v, op0=OP.bypass,
            op1=OP.mult, accum_out=sm["pv"][:]),
            reads=[pb_t, vb_t], writes=[SL, sm["pv"]])
        pg.op("vector", lambda e: e.tensor_mul(
            out=sm["Kc"][:], in0=sm["pv"][:], in1=sm["r"][:]),
            reads=[sm["pv"], sm["r"]], writes=[sm["Kc"]])
        pg.op("vector", lambda e, v=v, L=L: e.tensor_scalar_mul(
            out=qb_t[:Pp, 0:L], in0=v, scalar1=sm["Kc"][:]),
            reads=[vb_t, sm["Kc"]], writes=[qb_t])
        pg.op("vector", lambda e, L=L: e.tensor_tensor(
            out=qb_t[:Pp, 0:L], in0=pb_t[:Pp, 0:L], in1=qb_t[:Pp, 0:L],
            op=OP.subtract), reads=[pb_t, qb_t], writes=[qb_t])
        qrow_n = lambda n: qb_t[:Pp, 0:L].unsqueeze(1).to_broadcast([Pp, n, L])

        def vcol_r(r0, r1):
            return vb_t[:Pp, r0:r1].unsqueeze(2).to_broadcast(
                [Pp, r1 - r0, L])

        def qcol_r(r0, r1):
            return qb_t[:Pp, r0:r1].unsqueeze(2).to_broadcast(
                [Pp, r1 - r0, L])

        for (r0, r1, eng, rg) in (((0, Ls, "vector", (SL,)),) +
                                  (((Ls, L, "gpsimd", (SH,)),)
                                   if (split and Ls < L) else ())):
            aname = AL if eng == "vector" else AH
            o1o, o1a, o1b = srows(r0, r1), vcol_r(r0, r1), qrow_n(r1 - r0)
            pg.op(eng, lambda e, a=o1o, b=o1a, c=o1b: e.tensor_mul(
                out=a, in0=b, in1=c), reads=[vb_t, qb_t], writes=list(rg))
            s1m, s1s = mrows(r0, r1), srows(r0, r1)
            pg.op(eng, lambda e, a=s1m, b=s1s: e.tensor_tensor(
                out=a, in0=a, in1=b, op=OP.subtract),
                reads=[aname] + list(rg), writes=[aname])
            o2o, o2a, o2b = srows(r0, r1), qcol_r(r0, r1), vrow_n(r1 - r0)
            pg.op(eng, lambda e, a=o2o, b=o2a, c=o2b: e.tensor_mul(
                out=a, in0=b, in1=c), reads=[vb_t, qb_t], writes=list(rg))
            pg.op(eng, lambda e, a=s1m, b=s1s: e.tensor_tensor(
                out=a, in0=a, in1=b, op=OP.subtract),
                reads=[aname] + list(rg), writes=[aname])
        pg.op("scalar", lambda e, k=k: e.copy(
            out=eb_t[:Pp, k:k + 1], in_=sm["al"][:]),
            reads=[sm["al"]], writes=[eb_t])
    off = (m - 2) * m + (m - 1)
    pg.op("vector", lambda e, off=off: e.tensor_copy(
        out=eb_t[:Pp, m - 2:m - 1], in_=A[:, off:off + 1]),
        reads=[AL, AH], writes=[eb_t])


def emit_bisect(pg, A_t, db_t, eb_t, e2_t, ea_t, dms_t, qs_t, lo_t, hi_t, ht_t,
                sg_t, nu_t, mf_t, h0_t, jt_t, rr_t, out_t, Pp, m, ne, iters,
                a_off=0):
    """Sturm bisection: ne smallest eigenvalues (ascending) of the tridiagonal
    (diag of A_t flat matrix at a_off, off-diag eb_t)."""
    ne8 = ne * 8
    A = A_t[:Pp, a_off:a_off + m * m]
    dg = bass.AP(A.tensor, A.offset, [list(A.ap[0]), [m + 1, m]])
    pg.op("vector", lambda e: e.tensor_copy(out=db_t[:Pp, 0:m], in_=dg),
          reads=[A_t, A_t.name + "#lo", A_t.name + "#hi"], writes=[db_t])
    # e2n = -(e*e + 1e-30)   (negated so the Sturm step fuses into one stt)
    pg.op("vector", lambda e: e.tensor_mul(
        out=e2_t[:Pp, 0:m - 1], in0=eb_t[:Pp, 0:m - 1], in1=eb_t[:Pp, 0:m - 1]),
        reads=[eb_t], writes=[e2_t])
    pg.op("vector", lambda e: e.tensor_scalar(
        out=e2_t[:Pp, 0:m - 1], in0=e2_t[:Pp, 0:m - 1], scalar1=1e-30,
        scalar2=-1.0, op0=OP.add, op1=OP.mult),
        reads=[e2_t], writes=[e2_t])
    # gershgorin
    pg.op("scalar", lambda e: e.activation(
        out=ea_t[:Pp, 0:m - 1], in_=eb_t[:Pp, 0:m - 1], func=AF.Abs),
        reads=[eb_t], writes=[ea_t])
    pg.op("vector", lambda e: e.tensor_copy(out=sg_t[:Pp, 0:m], in_=db_t[:Pp, 0:m]),
          reads=[db_t], writes=[sg_t])
    pg.op("vector", lambda e: e.tensor_add(
        out=sg_t[:Pp, 0:m - 1], in0=sg_t[:Pp, 0:m - 1], in1=ea_t[:Pp, 0:m - 1]),
        reads=[sg_t, ea_t], writes=[sg_t])
    pg.op("vector", lambda e: e.tensor_add(
        out=sg_t[:Pp, 1:m], in0=sg_t[:Pp, 1:m], in1=ea_t[:Pp, 0:m - 1]),
        reads=[sg_t, ea_t], writes=[sg_t])
    pg.op("vector", lambda e: e.tensor_reduce(
        out=h0_t[:Pp, :], in_=sg_t[:Pp, 0:m], axis=AX.X, op=OP.max),
        reads=[sg_t], writes=[h0_t])
    pg.op("vector", lambda e: e.tensor_copy(
        out=hi_t[:Pp, 0:ne], in_=h0_t[:Pp, :].to_broadcast([Pp, ne])),
        reads=[h0_t], writes=[hi_t])
    pg.op("vector", lambda e: e.tensor_scalar_mul(
        out=lo_t[:Pp, 0:ne], in0=hi_t[:Pp, 0:ne], scalar1=-1.0 / 32.0),
        reads=[hi_t], writes=[lo_t])
    lo, hi, ht = lo_t[:Pp, 0:ne], hi_t[:Pp, 0:ne], ht_t[:Pp, 0:ne]
    sg = sg_t[:Pp, 0:ne8]
    for _ in range(iters):
        pg.op("vector", lambda e: e.tensor_sub(out=ht, in0=hi, in1=lo),
              reads=[hi_t, lo_t], writes=[ht_t])
        pg.op("vector", lambda e: e.tensor_scalar_mul(
            out=ht, in0=ht, scalar1=1.0 / 9.0), reads=[ht_t], writes=[ht_t])
        hbc = ht.unsqueeze(2).to_broadcast([Pp, ne, 8])
        lbc = lo.unsqueeze(2).to_broadcast([Pp, ne, 8])
        sg3 = sg.rearrange("p (a b) -> p a b", a=ne)
        rr3 = rr_t[:Pp, 0:ne8].rearrange("p (a b) -> p a b", a=ne)
        pg.op("vector", lambda e, hbc=hbc, sg3=sg3, rr3=rr3: e.tensor_mul(
            out=sg3, in0=rr3, in1=hbc),
            reads=[rr_t, ht_t], writes=[sg_t])
        pg.op("vector", lambda e, lbc=lbc, sg3=sg3: e.tensor_add(
            out=sg3, in0=sg3, in1=lbc), reads=[sg_t, lo_t], writes=[sg_t])
        dbc = db_t[:Pp, 0:m].unsqueeze(1).to_broadcast([Pp, ne8, m])
        sbc = sg.unsqueeze(2).to_broadcast([Pp, ne8, m])
        dmv = dms_t[:Pp, 0:ne8 * m].rearrange("p (a b) -> p a b", a=ne8)
        pg.op("vector", lambda e, dbc=dbc, sbc=sbc, dmv=dmv: e.tensor_tensor(
            out=dmv, in0=dbc, in1=sbc, op=OP.subtract),
            reads=[db_t, sg_t], writes=[dms_t])
        for i in range(m):
            qi = qs_t[:Pp, i * ne8:(i + 1) * ne8]
            di = bass.AP(dms_t[:Pp, :].tensor, dms_t[:Pp, :].offset + i,
                         [list(dms_t[:Pp, :].ap[0]), [m, ne8]])
            if i == 0:
                pg.op("scalar", lambda e, qi=qi, di=di: e.copy(
                    out=qi, in_=di), reads=[dms_t], writes=[qs_t])
            else:
                qp = qs_t[:Pp, (i - 1) * ne8:i * ne8]
                pg.op("vector", lambda e, qp=qp: e.reciprocal(out=sg, in_=qp),
                      reads=[qs_t], writes=[sg_t])
                # q_i = (u * e2n) + dms_i  (e2n = -(e^2+eps))
                pg.op("vector", lambda e, qi=qi, di=di, i=i:
                      e.scalar_tensor_tensor(
                          out=qi, in0=sg, scalar=e2_t[:Pp, i - 1:i], in1=di,
                          op0=OP.mult, op1=OP.add),
                      reads=[sg_t, e2_t, dms_t], writes=[qs_t])
        pg.op("vector", lambda e: e.tensor_scalar(
            out=qs_t[:Pp, 0:m * ne8], in0=qs_t[:Pp, 0:m * ne8], scalar1=0.0,
            scalar2=None, op0=OP.is_lt), reads=[qs_t], writes=[qs_t])
        qv = qs_t[:Pp, 0:m * ne8].rearrange("p (i r) -> p r i", i=m)
        pg.op("vector", lambda e, qv=qv: e.tensor_reduce(
            out=nu_t[:Pp, 0:ne8], in_=qv, axis=AX.X, op=OP.add),
            reads=[qs_t], writes=[nu_t])
        pg.op("vector", lambda e: e.tensor_tensor(
            out=nu_t[:Pp, 0:ne8], in0=nu_t[:Pp, 0:ne8], in1=jt_t[:Pp, 0:ne8],
            op=OP.is_lt), reads=[nu_t, jt_t], writes=[nu_t])
        nuv = nu_t[:Pp, 0:ne8].rearrange("p (a b) -> p a b", a=ne)
        pg.op("vector", lambda e, nuv=nuv: e.tensor_reduce(
            out=mf_t[:Pp, 0:ne], in_=nuv, axis=AX.X, op=OP.add),
            reads=[nu_t], writes=[mf_t])
        pg.op("vector", lambda e: e.tensor_mul(
            out=mf_t[:Pp, 0:ne], in0=mf_t[:Pp, 0:ne], in1=ht),
            reads=[mf_t, ht_t], writes=[mf_t])
        pg.op("vector", lambda e: e.tensor_add(
            out=lo, in0=lo, in1=mf_t[:Pp, 0:ne]),
            reads=[lo_t, mf_t], writes=[lo_t])
        pg.op("vector", lambda e: e.tensor_add(out=hi, in0=lo, in1=ht),
              reads=[lo_t, ht_t], writes=[hi_t])
    pg.op("vector", lambda e: e.tensor_add(
        out=out_t[:Pp, 0:ne], in0=lo, in1=hi), reads=[lo_t, hi_t],
        writes=[out_t])
    pg.op("vector", lambda e: e.tensor_scalar_mul(
        out=out_t[:Pp, 0:ne], in0=out_t[:Pp, 0:ne], scalar1=0.5),
        reads=[out_t], writes=[out_t])


# --------------------------------------------------------------- the program
def build_core_program(nc, dbg=False):
    cc = host_constants()
    dp = nc.declare_dram_parameter
    y_ext = dp("y", [BC, N_PTS], f32, isOutput=False)
    dc_ext = dp("dc", [BC * N_PTS, LIFT], f32, isOutput=False)
    base_ext = dp("base", [BC, K], i32, isOutput=False)
    nid_ext = dp("nid", [BC, S], f32, isOutput=False)
    patt_ext = dp("patt", [BC, 256], f32, isOutput=False)
    tri_ext = dp("tri", [BC, 256], f32, isOutput=False)
    se_ext = dp("Se", [128, 2 * E], f32, isOutput=False)
    g_ext = dp("G3", [E, 3 * T], f32, isOutput=False)
    m0_ext = dp("M0T", [E, 256], f32, isOutput=False)
    mu_ext = dp("Mu", [T, R * R], f32, isOutput=False)
    id_ext = dp("id128", [128, 128], f32, isOutput=False)
    rr16_ext = dp("rr16", [M16, 40], f32, isOutput=False)
    jt16_ext = dp("jt16", [M16, 40], f32, isOutput=False)
    rr105_ext = dp("rr105", [NPROB, 32], f32, isOutput=False)
    jt105_ext = dp("jt105", [NPROB, 32], f32, isOutput=False)
    w1_ext = dp("w1aug", [29, HID], f32, isOutput=False)
    w2a_ext = dp("w2a", [128, HID], f32, isOutput=False)
    w2b_ext = dp("w2b", [128, HID], f32, isOutput=False)
    w2c_ext = dp("w2c", [1, HID], f32, isOutput=False)
    out_ext = dp("out", [BC, HID], f32, isOutput=True)
    dbg_ext = {}
    if dbg:
        for nm, shp in [("d2", [BC, 256]), ("act", [BC, K]),
                        ("stats", [BC, 4]), ("W1", [E, NPROB]),
                        ("W2c0", [112, NPROB]), ("e16", [M16, 15]),
                        ("eig16", [M16, 5]), ("e105", [NPROB, 104]),
                        ("eigU", [NPROB, 4]), ("eig1", [NPROB, 4]),
                        ("utrow", [NPROB, R]), ("featsT", [29, BC]),
                        ("cand", [NPROB, 8])]:
            dbg_ext[nm] = dp("dbg_" + nm, shp, f32, isOutput=True)

    ctx = ExitStack()
    _ctr = [0]

    def sb(shape, dtype=f32):
        _ctr[0] += 1
        return ctx.enter_context(
            nc.sbuf_tensor(f"sb{_ctr[0]}", shape, dtype))

    def ps(shape):
        _ctr[0] += 1
        return ctx.enter_context(
            nc.psum_tensor(f"ps{_ctr[0]}", shape, f32))

    # big buffers (free-dim bytes add across ALL tiles; budget ~192KB/part)
    Ut = sb([NPROB, R * R])                    # 44.1KB
    scr = sb([NPROB, 3 * N_PTS])               # 48KB (topk views + tridiag scr)
    qs = sb([M16, max(R * 32, 40 * K)])        # 13.4KB
    dms = sb([M16, max(32 * R, 40 * K)])       # 13.4KB
    slab = [sb([112, 5 * MU_SLAB]) for _ in range(2)]   # 2x10.2KB
    # topk/d2 views live inside scr and Ut (dead before those are written)
    yt = scr[0:BC, 0:N_PTS]
    y2 = scr[0:BC, N_PTS:2 * N_PTS]
    y3 = scr[0:BC, 2 * N_PTS:3 * N_PTS]
    eqp = Ut[0:BC, 0:N_PTS]
    iotf = Ut[0:BC, N_PTS:2 * N_PTS]
    diffb = scr[0:BC, N_PTS:2 * N_PTS]   # reuses y2 slot (dead by then)

    # small tiles
    vals = sb([BC, K])
    idxf = sb([BC, K])
    idxi = sb([BC, K], i32)
    basei = sb([BC, K], i32)
    offs = sb([BC, K], i32)
    ppts = sb([BC, 256])
    d2 = sb([BC, 256])
    m2 = sb([BC, 256])
    Dm = sb([BC, 256])
    trim = sb([BC, 256])
    trif = sb([BC, 256])
    mask = sb([BC, K])
    nid = sb([BC, S])
    pattb = sb([BC, 256])
    amsk = sb([BC, 3 * 256])
    mtb = sb([BC, 256])
    stats = sb([BC, 4])
    s1 = sb([BC, 1])
    s2_ = sb([BC, 1])
    s3 = sb([BC, 1])
    seb = sb([128, 2 * E])
    g3b = sb([E, 3 * T])
    m0b = sb([E, 256])
    id128 = sb([128, 128])
    vecA = sb([128, 6 * BC])
    W1 = sb([E, NPROB])
    W2 = sb([112, 5 * NPROB])
    w2tmp = sb([112, BC])
    J16 = sb([M16, 256])
    e16 = sb([M16, 16])
    eig16 = sb([M16, 5])
    d16 = sb([M16, 16])
    e2_16 = sb([M16, 16])
    ea16 = sb([M16, 16])
    lo16 = sb([M16, 5])
    hi16 = sb([M16, 5])
    ht16 = sb([M16, 5])
    nu16 = sb([M16, 40])
    mf16 = sb([M16, 5])
    h016 = sb([M16, 1])
    rr16 = sb([M16, 40])
    jt16 = sb([M16, 40])
    e105 = sb([NPROB, 105])
    d105 = sb([NPROB, 105])
    e2105 = sb([NPROB, 105])
    ea105 = sb([NPROB, 105])
    eigU = sb([NPROB, 4])
    lo105 = sb([NPROB, 4])
    hi105 = sb([NPROB, 4])
    ht105 = sb([NPROB, 4])
    nu105 = sb([NPROB, 32])
    mf105 = sb([NPROB, 4])
    h0105 = sb([NPROB, 1])
    rr105 = sb([NPROB, 32])
    jt105 = sb([NPROB, 32])
    sg105 = sb([NPROB, max(105, 32)])
    sg16 = sb([M16, max(40, 16)])
    vb = sb([NPROB, R])
    avb = sb([NPROB, R])
    pb = sb([NPROB, R])
    qb = sb([NPROB, R])
    scr16 = sb([M16, 256])
    vb6 = sb([M16, K])
    avb6 = sb([M16, K])
    pb6 = sb([M16, K])
    qb6 = sb([M16, K])
    sm105 = {nm: sb([NPROB, 1]) for nm in
             ("s2", "sig", "x0", "sgn", "al", "vtv", "r", "r2", "pv", "Kc")}
    sm16 = {nm: sb([M16, 1]) for nm in
            ("s2", "sig", "x0", "sgn", "al", "vtv", "r", "r2", "pv", "Kc")}
    cand = sb([NPROB, 8])
    cneg = sb([NPROB, 8])
    csrt = sb([NPROB, 8])
    eig0a = sb([NPROB, 4])
    eig1a = sb([NPROB, 4])
    featsT = sb([29, BC])
    featrows = sb([BC, 28])
    hbuf = sb([BC, HID])
    hT0 = sb([128, BC])
    hT1 = sb([128, BC])
    ones1 = sb([1, BC])
    outs = sb([BC, HID])
    w1b = sb([29, HID])
    w2ab = sb([128, HID])
    w2bb = sb([128, HID])
    w2cb = sb([1, HID])

    # psum banks
    pJ16 = ps([128, 512])
    pA = ps([128, 512])
    pB = ps([128, 512])
    pU0 = ps([128, MU_SLAB])
    pU1 = ps([128, MU_SLAB])
    pM = ps([128, 512])

    pg = Prog(nc)
    V, SC, TE, GP, SY = "vector", "scalar", "tensor", "gpsimd", "sync"

    # ---- loads
    pg.dma(SY, yt, y_ext[:])
    pg.dma(SY, basei[:], base_ext[:])
    pg.dma(SY, nid[:], nid_ext[:])
    pg.dma(SY, pattb[:], patt_ext[:])
    pg.dma(SY, trim[:], tri_ext[:])
    pg.dma(SY, seb[:], se_ext[:])
    pg.dma(SY, g3b[:], g_ext[:])
    pg.dma(SY, m0b[:], m0_ext[:])
    pg.dma(SY, id128[:], id_ext[:])
    pg.dma(SY, rr16[:], rr16_ext[:])
    pg.dma(SY, jt16[:], jt16_ext[:])
    pg.dma(SY, rr105[:], rr105_ext[:])
    pg.dma(SY, jt105[:], jt105_ext[:])
    pg.dma(SY, w1b[:], w1_ext[:])
    pg.dma(SY, w2ab[:], w2a_ext[:])
    pg.dma(SY, w2bb[:], w2b_ext[:])
    pg.dma(SY, w2cb[:], w2c_ext[:])

    # ---- P1 topk (values + indices via match_replace/iota)
    pg.op(GP, lambda e: e.iota(iotf, pattern=[[1, N_PTS]], base=1,
                               channel_multiplier=0,
                               allow_small_or_imprecise_dtypes=True),
          writes=[Ut])
    pg.op(V, lambda e: e.max(out=vals[:, 0:8], in_=yt),
          reads=[scr], writes=[vals])
    pg.op(V, lambda e: e.match_replace(out=y2, in_to_replace=vals[:, 0:8],
                                       in_values=yt, imm_value=-3.0e38),
          reads=[scr, vals], writes=[scr])
    pg.op(V, lambda e: e.max(out=vals[:, 8:16], in_=y2),
          reads=[scr], writes=[vals])
    pg.op(V, lambda e: e.match_replace(out=y3, in_to_replace=vals[:, 8:16],
                                       in_values=y2, imm_value=-3.0e38),
          reads=[scr, vals], writes=[scr])
    pg.op(V, lambda e: e.tensor_tensor(out=eqp, in0=yt, in1=y2, op=OP.is_gt),
          reads=[scr], writes=[Ut])
    pg.op(V, lambda e: e.tensor_mul(out=eqp, in0=eqp, in1=iotf),
          reads=[Ut], writes=[Ut])
    pg.op(V, lambda e: e.max(out=idxf[:, 0:8], in_=eqp),
          reads=[Ut], writes=[idxf])
    pg.op(V, lambda e: e.tensor_tensor(out=eqp, in0=y2, in1=y3, op=OP.is_gt),
          reads=[scr], writes=[Ut])
    pg.op(V, lambda e: e.tensor_mul(out=eqp, in0=eqp, in1=iotf),
          reads=[Ut], writes=[Ut])
    pg.op(V, lambda e: e.max(out=idxf[:, 8:16], in_=eqp),
          reads=[Ut], writes=[idxf])
    pg.op(GP, lambda e: e.tensor_copy(out=idxi[:], in_=idxf[:]),
          reads=[idxf], writes=[idxi])
    pg.op(GP, lambda e: e.tensor_tensor(out=offs[:], in0=idxi[:], in1=basei[:],
                                        op=OP.add),
          reads=[idxi, basei], writes=[offs])
    # ---- P2 gather
    for j in range(K):
        pg.indirect(ppts[:, j * LIFT:(j + 1) * LIFT], dc_ext[:],
                    offs[:, j:j + 1])
    # ---- P3 d2 + mask
    p3 = ppts[:].rearrange("p (i l) -> p i l", i=K)
    xi = p3.unsqueeze(2).to_broadcast([BC, K, K, LIFT])
    xj = p3.unsqueeze(1).to_broadcast([BC, K, K, LIFT])
    dv = diffb.rearrange("p (a l) -> p a l", l=LIFT)
    dv4 = diffb.rearrange("p (i j l) -> p i j l", i=K, j=K)
    pg.op(V, lambda e: e.tensor_tensor(
        out=dv4, in0=xi, in1=xj, op=OP.subtract),
        reads=[ppts], writes=[scr])
    pg.op(SC, lambda e: e.square(out=diffb, in_=diffb), reads=[scr], writes=[scr])
    pg.op(V, lambda e: e.tensor_reduce(out=d2[:], in_=dv, axis=AX.X, op=OP.add),
          reads=[scr], writes=[d2])
    pg.op(V, lambda e: e.tensor_scalar(out=mask[:], in0=vals[:], scalar1=1e-3,
                                       scalar2=None, op0=OP.is_gt),
          reads=[vals], writes=[mask])
    mi = mask[:].unsqueeze(2).to_broadcast([BC, K, K])
    mj = mask[:].unsqueeze(1).to_broadcast([BC, K, K])
    pg.op(V, lambda e: e.tensor_tensor(
        out=m2[:].rearrange("p (i j) -> p i j", i=K), in0=mi, in1=mj,
        op=OP.mult), reads=[mask], writes=[m2])
    # ---- P4 stats
    pg.op(SC, lambda e: e.sqrt(out=Dm[:], in_=d2[:]), reads=[d2], writes=[Dm])
    pg.op(V, lambda e: e.tensor_scalar(out=trif[:], in0=d2[:], scalar1=0.0,
                                       scalar2=None, op0=OP.is_gt),
          reads=[d2], writes=[trif])
    pg.op(V, lambda e: e.tensor_mul(out=Dm[:], in0=Dm[:], in1=trif[:]),
          reads=[Dm, trif], writes=[Dm])
    pg.op(V, lambda e: e.tensor_mul(out=Dm[:], in0=Dm[:], in1=m2[:]),
          reads=[Dm, m2], writes=[Dm])
    pg.op(V, lambda e: e.tensor_mul(out=trif[:], in0=trim[:], in1=m2[:]),
          reads=[trim, m2], writes=[trif])
    pg.op(V, lambda e: e.tensor_reduce(out=s1[:], in_=trif[:], axis=AX.X,
                                       op=OP.add), reads=[trif], writes=[s1])
    pg.op(V, lambda e: e.tensor_scalar(out=s1[:], in0=s1[:], scalar1=1.0,
                                       scalar2=None, op0=OP.max),
          reads=[s1], writes=[s1])
    pg.op(V, lambda e: e.reciprocal(out=s1[:], in_=s1[:]),
          reads=[s1], writes=[s1])          # s1 = 1/tsum
    pg.op(V, lambda e: e.tensor_mul(out=trim[:], in0=Dm[:], in1=trif[:]),
          reads=[Dm, trif], writes=[trim])  # trim reused: D*tri
    pg.op(V, lambda e: e.tensor_reduce(out=s2_[:], in_=trim[:], axis=AX.X,
                                       op=OP.add), reads=[trim], writes=[s2_])
    pg.op(V, lambda e: e.tensor_mul(out=stats[:, 0:1], in0=s2_[:], in1=s1[:]),
          reads=[s2_, s1], writes=[stats])  # mean_d
    pg.op(V, lambda e: e.tensor_reduce(out=stats[:, 1:2], in_=trim[:],
                                       axis=AX.X, op=OP.max),
          reads=[trim], writes=[stats])     # max_d
    pg.op(V, lambda e: e.tensor_scalar(out=Dm[:], in0=Dm[:],
                                       scalar1=stats[:, 0:1], scalar2=None,
                                       op0=OP.subtract),
          reads=[Dm, stats], writes=[Dm])
    pg.op(SC, lambda e: e.square(out=Dm[:], in_=Dm[:]), reads=[Dm], writes=[Dm])
    pg.op(V, lambda e: e.tensor_mul(out=Dm[:], in0=Dm[:], in1=trif[:]),
          reads=[Dm, trif], writes=[Dm])
    pg.op(V, lambda e: e.tensor_reduce(out=s3[:], in_=Dm[:], axis=AX.X,
                                       op=OP.add), reads=[Dm], writes=[s3])
    pg.op(V, lambda e: e.tensor_mul(out=stats[:, 2:3], in0=s3[:], in1=s1[:]),
          reads=[s3, s1], writes=[stats])   # var_d
    pg.op(V, lambda e: e.tensor_scalar_add(out=s3[:], in0=stats[:, 1:2],
                                           scalar1=1e-6),
          reads=[stats], writes=[s3])
    pg.op(V, lambda e: e.reciprocal(out=s3[:], in_=s3[:]), reads=[s3],
          writes=[s3])
    pg.op(V, lambda e: e.tensor_mul(out=stats[:, 3:4], in0=stats[:, 0:1],
                                    in1=s3[:]),
          reads=[stats, s3], writes=[stats])  # comp
    # ---- P5 A_s, vecA (PE transposes)
    for s in range(S):
        asl = amsk[:, s * 256:(s + 1) * 256]
        pg.op(SC, lambda e, asl=asl, s=s: e.activation(
            out=asl, in_=d2[:], func=AF.Exp, scale=nid[:, s:s + 1]),
            reads=[d2, nid], writes=[amsk])
        pg.op(V, lambda e, asl=asl: e.tensor_mul(out=asl, in0=asl, in1=m2[:]),
              reads=[amsk, m2], writes=[amsk])
    for s in range(S):
        for c in range(2):
            asl = amsk[:, s * 256 + c * 128: s * 256 + (c + 1) * 128]
            pg.op(TE, lambda e, asl=asl: e.transpose(
                out=pA[0:128, 0:BC], in_=asl, identity=id128[0:BC, 0:BC]),
                reads=[amsk, id128], writes=[pA])
            dst = vecA[:, (2 * s + c) * BC:(2 * s + c + 1) * BC]
            pg.op(V, lambda e, dst=dst: e.tensor_copy(out=dst,
                                                      in_=pA[0:128, 0:BC]),
                  reads=[pA], writes=[vecA])
    # ---- P6 W1 = Se^T vecA  (per scale)
    for s in range(S):
        for c in range(2):
            va = vecA[:, (2 * s + c) * BC:(2 * s + c + 1) * BC]
            pg.op(TE, lambda e, va=va, c=c: e.matmul(
                out=pB[0:E, 0:BC], lhsT=seb[:, c * E:(c + 1) * E],
                rhs=va, start=(c == 0), stop=(c == 1)),
                reads=[seb, vecA], writes=[pB])
        pg.op(V, lambda e, s=s: e.tensor_copy(
            out=W1[:, s * BC:(s + 1) * BC], in_=pB[0:E, 0:BC]),
            reads=[pB], writes=[W1])
    # ---- P7 W2 (three gathers, product), chunked by 112
    for s in range(S):
        w1s = W1[:, s * BC:(s + 1) * BC]
        for c in range(5):
            for x in range(3):
                gsl = g3b[:, x * T + c * 112: x * T + (c + 1) * 112]
                pg.op(TE, lambda e, gsl=gsl, w1s=w1s, x=x: e.matmul(
                    out=pM[0:112, x * BC:(x + 1) * BC], lhsT=gsl, rhs=w1s,
                    start=True, stop=True),
                    reads=[g3b, W1], writes=[pM])
            pg.op(V, lambda e: e.tensor_copy(out=w2tmp[:], in_=pM[0:112, 0:BC]),
                  reads=[pM], writes=[w2tmp])
            pg.op(V, lambda e: e.tensor_mul(
                out=w2tmp[:], in0=w2tmp[:], in1=pM[0:112, BC:2 * BC]),
                reads=[w2tmp, pM], writes=[w2tmp])
            dst = W2[:, c * NPROB + s * BC: c * NPROB + (s + 1) * BC]
            pg.op(V, lambda e, dst=dst: e.tensor_mul(
                out=dst, in0=w2tmp[:], in1=pM[0:112, 2 * BC:3 * BC]),
                reads=[w2tmp, pM], writes=[W2])
    # ---- P8 L0 -> J16 rows 0:96 (via psum J16)
    for s in range(S):
        w1s = W1[:, s * BC:(s + 1) * BC]
        for c in range(2):
            pg.op(TE, lambda e, w1s=w1s, c=c: e.matmul(
                out=pB[0:128, 0:BC], lhsT=m0b[:, c * 128:(c + 1) * 128],
                rhs=w1s, start=True, stop=True),
                reads=[m0b, W1], writes=[pB])
            pg.op(V, lambda e: e.tensor_copy(out=hT0[:, 0:BC],
                                             in_=pB[0:128, 0:BC]),
                  reads=[pB], writes=[hT0])
            pg.op(TE, lambda e, s=s, c=c: e.matmul(
                out=pJ16[s * BC:(s + 1) * BC, c * 128:(c + 1) * 128],
                lhsT=hT0[:, 0:BC], rhs=id128[:, :], start=True, stop=True),
                reads=[hT0, id128], writes=[pJ16])
    pg.op(V, lambda e: e.tensor_copy(out=J16[0:NPROB, :], in_=pJ16[0:NPROB, 0:256]),
          reads=[pJ16], writes=[J16])
    # ---- P9 Mt -> J16 rows 96:128 (computed on partitions 0:32, DMA-moved)
    pg.op(SC, lambda e: e.sqrt(out=mask[:], in_=vals[:]),
          reads=[vals], writes=[mask])      # mask reused = sqrt(act)
    si = mask[:].unsqueeze(2).to_broadcast([BC, K, K])
    sj = mask[:].unsqueeze(1).to_broadcast([BC, K, K])
    pg.op(V, lambda e: e.tensor_tensor(
        out=mtb[:].rearrange("p (i j) -> p i j", i=K), in0=si, in1=sj,
        op=OP.mult), reads=[mask], writes=[mtb])
    pg.op(V, lambda e: e.tensor_mul(out=mtb[:], in0=mtb[:], in1=pattb[:]),
          reads=[mtb, pattb], writes=[mtb])
    pg.dma(SY, J16[NPROB:M16, :], mtb[:])
    # ---- P10/P11 tridiag16 + bisect16
    emit_tridiag(pg, J16[:], scr16, vb6, avb6, pb6, qb6, e16, sm16, M16, K, split=False)
    emit_bisect(pg, J16[:], d16, e16, e2_16, ea16, dms, qs, lo16, hi16, ht16,
                sg16, nu16, mf16, h016, jt16, rr16, eig16, M16, K, 5, BIS_IT16)
    # ---- P12 Ut assembly (stream Mu)
    nslab = (R * R + MU_SLAB - 1) // MU_SLAB
    for jj in range(nslab):
        w = min(MU_SLAB, R * R - jj * MU_SLAB)
        sl = slab[jj % 2]
        pu = pU0 if jj % 2 == 0 else pU1
        for c in range(5):
            pg.dma(SY, sl[:, c * MU_SLAB:c * MU_SLAB + w],
                   mu_ext[c * 112:(c + 1) * 112,
                          jj * MU_SLAB:jj * MU_SLAB + w])
        for c in range(5):
            pg.op(TE, lambda e, sl=sl, c=c, w=w, pu=pu: e.matmul(
                out=pu[0:NPROB, 0:w], lhsT=W2[:, c * NPROB:(c + 1) * NPROB],
                rhs=sl[:, c * MU_SLAB:c * MU_SLAB + w],
                start=(c == 0), stop=(c == 4)),
                reads=[W2, sl], writes=[pu])
        pg.op(V, lambda e, jj=jj, w=w, pu=pu: e.tensor_copy(
            out=Ut[0:NPROB, jj * MU_SLAB:jj * MU_SLAB + w],
            in_=pu[0:NPROB, 0:w]), reads=[pu], writes=[Ut])
    # ---- P13/P14 tridiag105 + bisect105
    emit_tridiag(pg, Ut[:], scr, vb, avb, pb, qb, e105, sm105, NPROB, R, split=True)
    emit_bisect(pg, Ut[:], d105, e105, e2105, ea105, dms, qs, lo105, hi105,
                ht105, sg105, nu105, mf105, h0105, jt105, rr105, eigU,
                NPROB, R, 4, BIS_IT105)
    # ---- P15 union merge + tau
    pg.op(V, lambda e: e.tensor_copy(out=cand[:, 0:4], in_=eigU[:, 0:4]),
          reads=[eigU], writes=[cand])
    for s in range(S):
        pg.dma(SY, cand[s * BC:(s + 1) * BC, 4:8], eig16[NPROB:M16, 1:5])
    pg.op(V, lambda e: e.tensor_scalar_mul(out=cneg[:], in0=cand[:],
                                           scalar1=-1.0),
          reads=[cand], writes=[cneg])
    pg.op(V, lambda e: e.max(out=csrt[:], in_=cneg[:]),
          reads=[cneg], writes=[csrt])
    pg.op(V, lambda e: e.tensor_scalar_mul(
        out=eig1a[:], in0=csrt[:, 0:4], scalar1=-1.0),
        reads=[csrt], writes=[eig1a])
    pg.op(V, lambda e: e.tensor_scalar_add(out=eig1a[:], in0=eig1a[:],
                                           scalar1=TAU),
          reads=[eig1a], writes=[eig1a])
    pg.op(V, lambda e: e.tensor_scalar_add(out=eig0a[:], in0=eig16[0:NPROB, 0:4],
                                           scalar1=TAU),
          reads=[eig16], writes=[eig0a])
    # ---- P16 feats: gather per-sample feature rows (DMA crosses partitions),
    # one PE transpose, then MLP
    for s in range(S):
        pg.dma(SY, featrows[0:BC, 8 * s:8 * s + 4],
               eig0a[s * BC:(s + 1) * BC, 0:4])
        pg.dma(SY, featrows[0:BC, 8 * s + 4:8 * s + 8],
               eig1a[s * BC:(s + 1) * BC, 0:4])
    pg.op(V, lambda e: e.tensor_copy(out=featrows[:, 24:28], in_=stats[:, 0:4]),
          reads=[stats], writes=[featrows])
    pg.op(TE, lambda e: e.transpose(
        out=pM[0:28, 0:BC], in_=featrows[:, 0:28], identity=id128[0:BC, 0:BC]),
        reads=[featrows, id128], writes=[pM])
    pg.op(V, lambda e: e.memset(featsT[0:29, :], 1.0), writes=[featsT])
    pg.op(V, lambda e: e.tensor_copy(out=featsT[0:28, :], in_=pM[0:28, 0:BC]),
          reads=[pM], writes=[featsT])
    pg.op(TE, lambda e: e.matmul(out=pB[0:BC, 0:HID], lhsT=featsT[:],
                                 rhs=w1b[:], start=True, stop=True),
          reads=[featsT, w1b], writes=[pB])
    pg.op(SC, lambda e: e.activation(out=hbuf[:], in_=pB[0:BC, 0:HID],
                                     func=AF.Gelu),
          reads=[pB], writes=[hbuf])
    for c, dst in ((0, hT0), (1, hT1)):
        pg.op(TE, lambda e, c=c: e.transpose(
            out=pA[0:128, 0:BC], in_=hbuf[:, c * 128:(c + 1) * 128],
            identity=id128[0:BC, 0:BC]),
            reads=[hbuf, id128], writes=[pA])
        pg.op(V, lambda e, dst=dst: e.tensor_copy(out=dst[:, 0:BC],
                                                  in_=pA[0:128, 0:BC]),
              reads=[pA], writes=[dst])
    pg.op(V, lambda e: e.memset(ones1[:], 1.0), writes=[ones1])
    pg.op(TE, lambda e: e.matmul(out=pM[0:BC, 0:HID], lhsT=hT0[:, 0:BC],
                                 rhs=w2ab[:], start=True, stop=False),
          reads=[hT0, w2ab], writes=[pM])
    pg.op(TE, lambda e: e.matmul(out=pM[0:BC, 0:HID], lhsT=hT1[:, 0:BC],
                                 rhs=w2bb[:], start=False, stop=False),
          reads=[hT1, w2bb], writes=[pM])
    pg.op(TE, lambda e: e.matmul(out=pM[0:BC, 0:HID], lhsT=ones1[:],
                                 rhs=w2cb[:], start=False, stop=True),
          reads=[ones1, w2cb], writes=[pM])
    pg.op(V, lambda e: e.tensor_copy(out=outs[:], in_=pM[0:BC, 0:HID]),
          reads=[pM], writes=[outs])
    pg.dma(SY, out_ext[:], outs[:])

    if dbg:
        pg.dma(SY, dbg_ext["d2"][:], d2[:])
        pg.dma(SY, dbg_ext["act"][:], vals[:])
        pg.dma(SY, dbg_ext["stats"][:], stats[:])
        pg.dma(SY, dbg_ext["W1"][:], W1[:])
        pg.dma(SY, dbg_ext["W2c0"][:], W2[:, 0:NPROB])
        pg.dma(SY, dbg_ext["e16"][:], e16[:, 0:15])
        pg.dma(SY, dbg_ext["eig16"][:], eig16[:])
        pg.dma(SY, dbg_ext["e105"][:], e105[:, 0:104])
        pg.dma(SY, dbg_ext["eigU"][:], eigU[:])
        pg.dma(SY, dbg_ext["eig1"][:], eig1a[:])
        pg.dma(SY, dbg_ext["utrow"][:], Ut[0:NPROB, 0:R])
        pg.dma(SY, dbg_ext["featsT"][:], featsT[:])
        pg.dma(SY, dbg_ext["cand"][:], cand[:])
    pg.build()
    ctx.close()
    return nc


# ----------------------------------------------------------------- host API
_NC_CACHE = {}


def _get_nc(dbg=False):
    if dbg not in _NC_CACHE:
        nc = bass.Bass()
        build_core_program(nc, dbg=dbg)
        _NC_CACHE[dbg] = nc
    return _NC_CACHE[dbg]


def make_in_maps(dense_cloud, y_star, log_scales, w1, b1, w2, b2, dbg=False):
    cc = host_constants()
    nid = np.tile((-1.0 / (2.0 * np.exp(log_scales) ** 2 + 1e-8)
                   ).astype(np.float32)[None, :], (BC, 1))
    w1aug = np.concatenate([w1, b1[None, :]], 0).astype(np.float32)
    w2aug = np.concatenate([w2, b2[None, :]], 0).astype(np.float32)
    shared = {"nid": nid, "patt": cc["patt"], "tri": cc["tri"], "Se": np.concatenate([cc["Se"][0:128], cc["Se"][128:256]], 1),
              "G3": np.ascontiguousarray(np.swapaxes(cc["G"], 0, 1).reshape(E, 3 * T)), "M0T": cc["M0T"],
              "Mu": cc["Mu"], "id128": cc["id128"], "base": cc["base"],
              "rr16": cc["rr16"], "jt16": cc["jt16"], "rr105": cc["rr105"],
              "jt105": cc["jt105"], "w1aug": w1aug,
              "w2a": w2aug[0:128], "w2b": w2aug[128:256],
              "w2c": w2aug[256:257]}
    in_maps = []
    for i in range(NCORES):
        m = dict(shared)
        m["y"] = np.ascontiguousarray(y_star[i * BC:(i + 1) * BC])
        m["dc"] = np.ascontiguousarray(
            dense_cloud[i * BC:(i + 1) * BC].reshape(BC * N_PTS, LIFT))
        in_maps.append(m)
    return in_maps


def kernel(dense_cloud, y_star, log_scales, w1, b1, w2, b2,
           B1=None, B2=None, e_i=None, e_j=None, t_ij=None, t_jk=None,
           t_ik=None, **extra):
    dense_cloud = np.asarray(dense_cloud, np.float32)
    y_star = np.asarray(y_star, np.float32)
    in_maps = make_in_maps(dense_cloud, y_star, np.asarray(log_scales),
                           np.asarray(w1), np.asarray(b1), np.asarray(w2),
                           np.asarray(b2))
    nc = _get_nc(dbg=False)
    res = run_bass_kernel_spmd(nc, in_maps, list(range(NCORES))).results
    return np.concatenate([r["out"] for r in res], 0).astype(
        dense_cloud.dtype)


if __name__ == "__main__":
    # framework smoke test
    nc = bass.Bass()
    P, N = 32, 64
    a_ext = nc.declare_dram_parameter("a", [P, N], f32, isOutput=False)
    b_ext = nc.declare_dram_parameter("b", [P, N], f32, isOutput=False)
    o_ext = nc.declare_dram_parameter("o", [P, N], f32, isOutput=True)
    with nc.sbuf_tensor([P, N], f32) as at, nc.sbuf_tensor([P, N], f32) as bt, \
         nc.sbuf_tensor([P, N], f32) as b2t, nc.sbuf_tensor([P, N], f32) as ot:
        pg = Prog(nc)
        pg.dma("sync", at[:], a_ext[:])
        pg.dma("sync", bt[:], b_ext[:])
        pg.op("scalar", lambda e: e.mul(out=b2t[:], in_=bt[:], mul=2.0),
              reads=[bt[:]], writes=[b2t[:]])
        pg.op("vector", lambda e: e.tensor_add(out=ot[:], in0=at[:], in1=b2t[:]),
              reads=[at[:], b2t[:]], writes=[ot[:]])
        pg.dma("sync", o_ext[:], ot[:])
        pg.build()
    a = np.random.rand(P, N).astype(np.float32)
    b = np.random.rand(P, N).astype(np.float32)
    res = run_bass_kernel_spmd(nc, [{"a": a, "b": b}], [0]).results[0]
    print("smoke ok:", np.allclose(res["o"], a + 2 * b))
